# revision 1
# baseline (speedup 1.0000x reference)
"""Trainium2 Bass kernel for nn_Block_74363063763569 (BEiT-style transformer block).

Data-parallel over batch across 8 NeuronCores (8 elems/core), zero collectives.
Self-contained: builds, compiles (cached) and runs the Bass kernel via
run_bass_kernel_spmd on cores 0-7.
"""
import sys, json
sys.path.insert(0, "/opt/trn_rl_repo")
import numpy as np


def _legalize_waits(bir_bytes, max_waits=1):
    """This container's walrus rejects >1 sync wait per instruction; split
    extras into preceding single-wait EventSemaphore instructions."""
    j = json.loads(bir_bytes)
    for f in j["functions"]:
        for b in f["blocks"]:
            out = []
            for inst in b["instructions"]:
                si = inst.get("sync_info")
                waits = si.get("on_wait", []) if si else []
                if len(waits) > max_waits:
                    keep, extra = waits[:max_waits], waits[max_waits:]
                    for k, w in enumerate(extra):
                        out.append({"debug": inst.get("debug", 0), "engine": inst["engine"],
                                    "ins": [], "name": f"{inst['name']}_w{k}",
                                    "opcode": "EventSemaphore", "outs": [],
                                    "sync_info": {"on_update": [], "on_wait": [w]}})
                    si["on_wait"] = keep
                out.append(inst)
            b["instructions"] = out
    return json.dumps(j).encode()


"""Bass/Tile kernel builder for the BEiT-style transformer block.

Strategy (per core, data-parallel over batch):
- 8 batch elements per core, processed as 4 pairs (token axis packed to 394).
- Residual stream kept in NORMAL layout [tokens, features] fp32.
- Matmul activations in TRANSPOSED layout [features, tokens] bf16
  (produced via PE transposes of the LN outputs).
- LN affine folded into qkv/fc1 weights; gamma1/gamma2 folded into
  proj/fc2 weights; attention scale folded into q weights (host side).
- Attention: logits computed transposed [t_k, t_q]; softmax without
  max-subtraction (logits are provably small); denominator via a ones
  column appended to V; per-column normalization via GPSIMD
  partition_broadcast of the reciprocal row.
"""

import numpy as np

import concourse.bass as bass
import concourse.tile as tile
import concourse.mybir as mybir
from concourse.masks import make_identity

FP32 = mybir.dt.float32
BF16 = mybir.dt.bfloat16

B = 64
N = 197
C = 768
H = 12
D = 64
HID = 3072
NCORES = 8
BPC = B // NCORES          # 8 batch elems per core
NPAIRS_FULL = BPC // 2     # 4
KT = C // 128              # 6 k-tiles of 128 over features
KT2 = HID // 128           # 24 k-tiles over hidden
LN_EPS = 1e-5

# token tiling: 197 = 128 + 69
T_TILES = [(0, 128), (128, 69)]
# output chunks over feature dim 768 = 512 + 256
C_CHUNKS = [(0, 512), (512, 256)]

AL = mybir.AluOpType
AF = mybir.ActivationFunctionType


def build_nc(npairs=NPAIRS_FULL):
    nb = 2 * npairs
    nc = bass.Bass()

    x_d = nc.dram_tensor("x", [nb, N, C], FP32, kind="ExternalInput")
    qkvT_d = nc.dram_tensor("qkvT", [C, 3 * C], BF16, kind="ExternalInput")
    projT_d = nc.dram_tensor("projT", [C, C], BF16, kind="ExternalInput")
    fc1T_d = nc.dram_tensor("fc1T", [C, HID], BF16, kind="ExternalInput")
    fc2T_d = nc.dram_tensor("fc2T", [HID, C], BF16, kind="ExternalInput")
    rpb0_d = nc.dram_tensor("rpb0", [128, H, N], BF16, kind="ExternalInput")
    rpb1_d = nc.dram_tensor("rpb1", [69, H, N], BF16, kind="ExternalInput")
    qb_d = nc.dram_tensor("qb", [C], FP32, kind="ExternalInput")
    kb_d = nc.dram_tensor("kb", [C], FP32, kind="ExternalInput")
    fc1b_d = nc.dram_tensor("fc1b", [HID], FP32, kind="ExternalInput")
    vb_d = nc.dram_tensor("vbrow", [C], BF16, kind="ExternalInput")
    pb_d = nc.dram_tensor("pbrow", [C], BF16, kind="ExternalInput")
    f2b_d = nc.dram_tensor("f2brow", [C], BF16, kind="ExternalInput")
    y_d = nc.dram_tensor("y", [nb, N, C], FP32, kind="ExternalOutput")

    with tile.TileContext(nc) as tc:
        with (
            tc.tile_pool(name="singles", bufs=1) as singles,
            tc.tile_pool(name="resid", bufs=1) as resid,     # x0/x1/out fp32
            tc.tile_pool(name="b394", bufs=1) as b394,      # bf16 [128,394] transient
            tc.tile_pool(name="xn", bufs=3) as xnp,
            tc.tile_pool(name="vpool", bufs=4) as vpool,
            tc.tile_pool(name="expp", bufs=4) as expp,
            tc.tile_pool(name="dpool", bufs=2) as dpool,
            tc.tile_pool(name="small", bufs=8) as small,
            tc.tile_pool(name="ps_tr", bufs=2, space="PSUM") as ps_tr,
            tc.tile_pool(name="ps_mm", bufs=2, space="PSUM") as ps_mm,
            tc.tile_pool(name="ps_at", bufs=4, space="PSUM") as ps_at,
        ):
            # ---- persistent weights / constants ----
            qkvT = [singles.tile([128, 3 * C], BF16, tag=f"qkvT{k}", name=f"qkvT{k}") for k in range(KT)]
            projT = [singles.tile([128, C], BF16, tag=f"projT{k}", name=f"projT{k}") for k in range(KT)]
            fc1T = [singles.tile([128, HID], BF16, tag=f"fc1T{k}", name=f"fc1T{k}") for k in range(KT)]
            fc2T = [singles.tile([128, C], BF16, tag=f"fc2T{k}", name=f"fc2T{k}") for k in range(KT2)]
            rpb0 = singles.tile([128, H, N], BF16, tag="rpb0")
            rpb1 = singles.tile([69, H, N], BF16, tag="rpb1")
            qb_sb = singles.tile([128, KT], FP32, tag="qb")
            kb_sb = singles.tile([128, KT], FP32, tag="kb")
            fc1b_sb = singles.tile([128, KT2], FP32, tag="fc1b")
            vb_sb = singles.tile([1, C], BF16, tag="vb")
            pb_sb = singles.tile([1, C], BF16, tag="pb")
            f2b_sb = singles.tile([1, C], BF16, tag="f2b")
            ident = singles.tile([128, 128], BF16, tag="ident")
            ones_row = singles.tile([1, 128], BF16, tag="ones")
            ones_f32 = singles.tile([1, 128], FP32, tag="ones32")
            eps_sb = singles.tile([128, 1], FP32, tag="eps")

            for k in range(KT):
                nc.sync.dma_start(qkvT[k][:], qkvT_d[k * 128:(k + 1) * 128, :])
            for k in range(KT):
                nc.sync.dma_start(projT[k][:], projT_d[k * 128:(k + 1) * 128, :])
            for k in range(KT):
                nc.sync.dma_start(fc1T[k][:], fc1T_d[k * 128:(k + 1) * 128, :])
            for k in range(KT2):
                nc.sync.dma_start(fc2T[k][:], fc2T_d[k * 128:(k + 1) * 128, :])
            nc.sync.dma_start(rpb0[:], rpb0_d[:])
            nc.sync.dma_start(rpb1[:], rpb1_d[:])
            nc.sync.dma_start(qb_sb[:], qb_d[:].rearrange("(k p) -> p k", p=128))
            nc.sync.dma_start(kb_sb[:], kb_d[:].rearrange("(k p) -> p k", p=128))
            nc.sync.dma_start(fc1b_sb[:], fc1b_d[:].rearrange("(k p) -> p k", p=128))
            nc.sync.dma_start(vb_sb[:], vb_d[None, :])
            nc.sync.dma_start(pb_sb[:], pb_d[None, :])
            nc.sync.dma_start(f2b_sb[:], f2b_d[None, :])
            make_identity(nc, ident[:])
            nc.vector.memset(ones_row[:], 1.0)
            nc.vector.memset(ones_f32[:], 1.0)
            nc.vector.memset(eps_sb[:], LN_EPS)

            def ln_transpose(x_tiles, tag, out_tags):
                """LN over feature dim + PE-transpose into pair-packed [128, 2N] bf16 tiles."""
                xT = [b394.tile([128, 2 * N], BF16, tag=out_tags[k], name=f"{tag}T{k}")
                      for k in range(KT)]
                for (e, j), xt in x_tiles.items():
                    toff, tcnt = T_TILES[j]
                    stats = small.tile([128, 3, 6], FP32, tag=f"st_{tag}")
                    mv = small.tile([128, 2], FP32, tag=f"mv_{tag}")
                    sd = small.tile([128, 1], FP32, tag=f"sd_{tag}")
                    rstd = small.tile([128, 1], FP32, tag=f"rs_{tag}")
                    for g in range(3):
                        nc.vector.bn_stats(stats[:tcnt, g, :], xt[:tcnt, g * 256:(g + 1) * 256])
                    nc.vector.bn_aggr(mv[:tcnt], stats[:tcnt])
                    nc.scalar.activation(sd[:tcnt], mv[:tcnt, 1:2], AF.Ln, bias=eps_sb[:tcnt])
                    nc.scalar.activation(rstd[:tcnt], sd[:tcnt], AF.Exp, scale=-0.5)
                    xn = xnp.tile([128, C], BF16, tag="xn")
                    nc.vector.tensor_scalar(
                        xn[:tcnt, :], xt[:tcnt, :],
                        mv[:tcnt, 0:1], rstd[:tcnt, 0:1],
                        op0=AL.subtract, op1=AL.mult)
                    for cb in range(KT):
                        pt = ps_tr.tile([128, 128], BF16, tag="ps_tr")
                        nc.tensor.transpose(
                            pt[:128, :tcnt],
                            xn[:tcnt, cb * 128:(cb + 1) * 128],
                            ident[:tcnt, :tcnt])
                        nc.vector.tensor_copy(
                            xT[cb][:, e * N + toff: e * N + toff + tcnt],
                            pt[:128, :tcnt])
                return xT

            # t-slice within the packed [2N] axis for (e, j)
            def tslice(e, j):
                toff, tcnt = T_TILES[j]
                return e * N + toff, tcnt

            for s in range(npairs):
                # ---------------- load x0 ----------------
                x0 = {}
                for e in range(2):
                    bidx = 2 * s + e
                    for j, (toff, tcnt) in enumerate(T_TILES):
                        t = resid.tile([128, C], FP32, tag=f"x0_{e}{j}", bufs=2 if e == 0 else 1)
                        nc.scalar.dma_start(t[:tcnt, :], x_d[bidx, toff:toff + tcnt, :])
                        x0[(e, j)] = t

                # ---------------- LN1 + transpose ----------------
                xnT = ln_transpose(x0, "ln1", [f"b394_xnT{k}" for k in range(KT)])

                # ---------------- qT, kT ----------------
                qT = [b394.tile([128, 2 * N], BF16, tag=f"b394_qT{ob}", name=f"qT{ob}") for ob in range(KT)]
                kT = [b394.tile([128, 2 * N], BF16, tag=f"b394_kT{ob}", name=f"kT{ob}") for ob in range(KT)]
                for dst, base, bias in ((qT, 0, qb_sb), (kT, C, kb_sb)):
                    for ob in range(KT):
                        ps = ps_mm.tile([128, 2 * N], FP32, tag="ps_mm")
                        for k in range(KT):
                            nc.tensor.matmul(
                                ps[:, :], qkvT[k][:, base + ob * 128: base + (ob + 1) * 128],
                                xnT[k][:, :], start=(k == 0), stop=(k == KT - 1))
                        nc.vector.tensor_scalar_add(dst[ob][:, :], ps[:, :], bias[:, ob:ob + 1])

                # ---------------- v (normal layout, per elem/t-tile) ----------------
                v_sb = {}
                for e in range(2):
                    for j, (toff, tcnt) in enumerate(T_TILES):
                        vt = vpool.tile([128, H, D + 1], BF16, tag="v")
                        nc.vector.memset(vt[:, :, D:D + 1], 1.0)
                        ts_off, ts_cnt = tslice(e, j)
                        for ci, (coff, csz) in enumerate(C_CHUNKS):
                            ps = ps_mm.tile([128, 512], FP32, tag="ps_mm")
                            for k in range(KT):
                                nc.tensor.matmul(
                                    ps[:ts_cnt, :csz],
                                    xnT[k][:, ts_off:ts_off + ts_cnt],
                                    qkvT[k][:, 2 * C + coff: 2 * C + coff + csz],
                                    start=(k == 0), stop=False)
                            nc.tensor.matmul(
                                ps[:ts_cnt, :csz],
                                ones_row[0:1, :ts_cnt],
                                vb_sb[0:1, coff:coff + csz],
                                start=False, stop=True)
                            h0 = coff // D
                            nh = csz // D
                            nc.vector.tensor_copy(
                                vt[:ts_cnt, h0:h0 + nh, 0:D],
                                ps[:ts_cnt, :csz])
                        v_sb[(e, j)] = vt

                # ---------------- attention ----------------
                aT = [b394.tile([128, 2 * N], BF16, tag=f"b394_aT{cb}", name=f"aT{cb}") for cb in range(KT)]
                rpb = (rpb0, rpb1)
                for e in range(2):
                    for h in range(H):
                        hp, hi = divmod(h, 2)
                        rbase = 64 * hi
                        exp_t = []
                        for j2, (tkoff, tkcnt) in enumerate(T_TILES):
                            L = ps_at.tile([128, N], FP32, tag="ps_at")
                            # logitsT[tk, tq] = k_h[tk,:] . q_h[tq,:]
                            nc.tensor.matmul(
                                L[:tkcnt, :N],
                                kT[hp][rbase:rbase + 64, e * N + tkoff: e * N + tkoff + tkcnt],
                                qT[hp][rbase:rbase + 64, e * N: e * N + N],
                                start=True, stop=False)
                            # += rpbT via identity matmul
                            nc.tensor.matmul(
                                L[:tkcnt, :N],
                                ident[:tkcnt, :tkcnt],
                                rpb[j2][:tkcnt, h, :],
                                start=False, stop=True)
                            et = expp.tile([128, N], BF16, tag="exp")
                            nc.scalar.activation(et[:tkcnt, :], L[:tkcnt, :N], AF.Exp)
                            exp_t.append(et)
                        O = ps_at.tile([D + 1, N], FP32, tag="ps_at")
                        for j2, (tkoff, tkcnt) in enumerate(T_TILES):
                            nc.tensor.matmul(
                                O[:D + 1, :N],
                                v_sb[(e, j2)][:tkcnt, h, :],
                                exp_t[j2][:tkcnt, :N],
                                start=(j2 == 0), stop=(j2 == 1))
                        lden = small.tile([1, N], FP32, tag="lden", name="lden")
                        r = small.tile([1, N], FP32, tag="recip", name="r")
                        nc.scalar.activation(lden[:, :], O[D:D + 1, :N], AF.Ln)
                        nc.scalar.activation(r[:, :], lden[:, :], AF.Exp, scale=-1.0)
                        Dn = ps_at.tile([64, N], FP32, tag="ps_at")
                        nc.tensor.matmul(Dn[:, :], ones_f32[0:1, 0:64], r[0:1, :])
                        Dsb = dpool.tile([64, N], FP32, tag="D")
                        nc.scalar.copy(Dsb[:, :], Dn[:, :])
                        nc.vector.tensor_tensor(
                            aT[hp][rbase:rbase + 64, e * N: e * N + N],
                            O[0:D, :N], Dsb[:, :], op=AL.mult)

                # ---------------- proj + residual -> x1 ----------------
                x1 = {}
                for e in range(2):
                    for j, (toff, tcnt) in enumerate(T_TILES):
                        xt = resid.tile([128, C], FP32, tag=f"x1_{e}{j}")
                        ts_off, ts_cnt = tslice(e, j)
                        for ci, (coff, csz) in enumerate(C_CHUNKS):
                            ps = ps_mm.tile([128, 512], FP32, tag="ps_mm")
                            for k in range(KT):
                                nc.tensor.matmul(
                                    ps[:ts_cnt, :csz],
                                    aT[k][:, ts_off:ts_off + ts_cnt],
                                    projT[k][:, coff:coff + csz],
                                    start=(k == 0), stop=False)
                            nc.tensor.matmul(
                                ps[:ts_cnt, :csz],
                                ones_row[0:1, :ts_cnt],
                                pb_sb[0:1, coff:coff + csz],
                                start=False, stop=True)
                            nc.vector.tensor_tensor(
                                xt[:ts_cnt, coff:coff + csz],
                                ps[:ts_cnt, :csz],
                                x0[(e, j)][:ts_cnt, coff:coff + csz], op=AL.add)
                        x1[(e, j)] = xt

                # ---------------- LN2 + transpose ----------------
                hnT = ln_transpose(x1, "ln2", [f"b394_hnT{k}" for k in range(KT)])

                # ---------------- fc1 + gelu -> hT ----------------
                _ht_tags = ([f"b394_xnT{k}" for k in range(KT)] + [f"b394_qT{k}" for k in range(KT)] + [f"b394_kT{k}" for k in range(KT)] + [f"b394_aT{k}" for k in range(KT)])
                hT = [b394.tile([128, 2 * N], BF16, tag=_ht_tags[ob], name=f"hT{ob}") for ob in range(KT2)]
                for ob in range(KT2):
                    ps = ps_mm.tile([128, 2 * N], FP32, tag="ps_mm")
                    for k in range(KT):
                        nc.tensor.matmul(
                            ps[:, :], fc1T[k][:, ob * 128:(ob + 1) * 128],
                            hnT[k][:, :], start=(k == 0), stop=(k == KT - 1))
                    nc.scalar.activation(
                        hT[ob][:, :], ps[:, :], AF.Gelu,
                        bias=fc1b_sb[:, ob:ob + 1])

                # ---------------- fc2 + residual -> y ----------------
                for e in range(2):
                    bidx = 2 * s + e
                    for j, (toff, tcnt) in enumerate(T_TILES):
                        ot = resid.tile([128, C], FP32, tag=f"x0_{e}{j}", name=f"out_{e}{j}", bufs=2 if e == 0 else 1)
                        ts_off, ts_cnt = tslice(e, j)
                        for ci, (coff, csz) in enumerate(C_CHUNKS):
                            ps = ps_mm.tile([128, 512], FP32, tag="ps_mm")
                            for k in range(KT2):
                                nc.tensor.matmul(
                                    ps[:ts_cnt, :csz],
                                    hT[k][:, ts_off:ts_off + ts_cnt],
                                    fc2T[k][:, coff:coff + csz],
                                    start=(k == 0), stop=False)
                            nc.tensor.matmul(
                                ps[:ts_cnt, :csz],
                                ones_row[0:1, :ts_cnt],
                                f2b_sb[0:1, coff:coff + csz],
                                start=False, stop=True)
                            nc.vector.tensor_tensor(
                                ot[:ts_cnt, coff:coff + csz],
                                ps[:ts_cnt, :csz],
                                x1[(e, j)][:ts_cnt, coff:coff + csz], op=AL.add)
                        nc.gpsimd.dma_start(y_d[bidx, toff:toff + tcnt, :], ot[:tcnt, :])

    return nc


def fold_weights(inputs):
    """Host-side folding. Returns dict of per-core-shared input arrays."""
    import ml_dtypes
    f32 = np.float32
    bf16 = ml_dtypes.bfloat16
    g = {k: np.asarray(v) for k, v in inputs.items()}
    n1w, n1b = g["n1_w"].astype(f32), g["n1_b"].astype(f32)
    n2w, n2b = g["n2_w"].astype(f32), g["n2_b"].astype(f32)
    g1, g2 = g["gamma1"].astype(f32), g["gamma2"].astype(f32)
    qkv_w = g["qkv_w"].astype(f32)
    q_bias, v_bias = g["q_bias"].astype(f32), g["v_bias"].astype(f32)
    proj_w, proj_b = g["proj_w"].astype(f32), g["proj_b"].astype(f32)
    fc1_w, fc1_b = g["fc1_w"].astype(f32), g["fc1_b"].astype(f32)
    fc2_w, fc2_b = g["fc2_w"].astype(f32), g["fc2_b"].astype(f32)

    qkv_bias = np.concatenate([q_bias, np.zeros_like(q_bias), v_bias])
    Wq = qkv_w * n1w[None, :]
    bq = qkv_bias + qkv_w @ n1b
    scale = (C // H) ** -0.5
    Wq[:C] *= scale
    bq[:C] *= scale

    Pw = g1[:, None] * proj_w
    pb = g1 * proj_b
    F1 = fc1_w * n2w[None, :]
    f1b = fc1_b + fc1_w @ n2b
    F2 = g2[:, None] * fc2_w
    f2b = g2 * fc2_b

    table = g["rel_bias_table"].astype(f32)
    idx = np.asarray(g["rel_index"]).reshape(-1)
    rpb_ref = table[idx].reshape(N, N, H).transpose(2, 0, 1)  # [h, tq, tk]
    rpbT = rpb_ref.transpose(0, 2, 1)                          # [h, tk, tq]
    rpb0 = np.ascontiguousarray(rpbT[:, :128, :].transpose(1, 0, 2)).astype(bf16)
    rpb1 = np.ascontiguousarray(rpbT[:, 128:, :].transpose(1, 0, 2)).astype(bf16)

    return {
        "qkvT": np.ascontiguousarray(Wq.T).astype(bf16),
        "projT": np.ascontiguousarray(Pw.T).astype(bf16),
        "fc1T": np.ascontiguousarray(F1.T).astype(bf16),
        "fc2T": np.ascontiguousarray(F2.T).astype(bf16),
        "rpb0": rpb0,
        "rpb1": rpb1,
        "qb": np.ascontiguousarray(bq[:C]),
        "kb": np.ascontiguousarray(bq[C:2 * C]),
        "fc1b": f1b,
        "vbrow": bq[2 * C:].astype(bf16),
        "pbrow": pb.astype(bf16),
        "f2brow": f2b.astype(bf16),
    }


_CACHE = {}


def _get_nc():
    if "nc" not in _CACHE:
        nc = build_nc()
        patched = _legalize_waits(nc.to_json_bytes())
        nc.to_json_bytes = lambda: patched
        _CACHE["nc"] = nc
    return _CACHE["nc"]


def kernel(**inputs):
    from concourse.bass_utils import run_bass_kernel_spmd
    nc = _get_nc()
    folded = fold_weights(inputs)
    x = np.ascontiguousarray(np.asarray(inputs["x"], dtype=np.float32))
    assert x.shape == (B, N, C), x.shape
    in_maps = []
    for c in range(NCORES):
        m = dict(folded)
        m["x"] = np.ascontiguousarray(x[c * BPC:(c + 1) * BPC])
        in_maps.append(m)
    res = run_bass_kernel_spmd(nc, in_maps, core_ids=list(range(NCORES)))
    out = np.concatenate([res.results[c]["y"] for c in range(NCORES)], axis=0)
    return out.astype(np.float32)



# revision 10
# speedup vs baseline: 1.2541x; 1.2541x over previous
"""Trainium2 Bass kernel for nn_Block_74363063763569 (BEiT-style transformer block).

Data-parallel over batch across 8 NeuronCores (8 elems/core), zero collectives.
Self-contained: builds, compiles (cached) and runs the Bass kernel via
run_bass_kernel_spmd on cores 0-7.
"""
import sys, json
sys.path.insert(0, "/opt/trn_rl_repo")
import numpy as np


def _legalize_waits(bir_bytes, max_waits=1):
    """This container's walrus rejects >1 sync wait per instruction; split
    extras into preceding single-wait EventSemaphore instructions."""
    j = json.loads(bir_bytes)
    for f in j["functions"]:
        for b in f["blocks"]:
            out = []
            for inst in b["instructions"]:
                si = inst.get("sync_info")
                waits = si.get("on_wait", []) if si else []
                if len(waits) > max_waits:
                    keep, extra = waits[:max_waits], waits[max_waits:]
                    for k, w in enumerate(extra):
                        out.append({"debug": inst.get("debug", 0), "engine": inst["engine"],
                                    "ins": [], "name": f"{inst['name']}_w{k}",
                                    "opcode": "EventSemaphore", "outs": [],
                                    "sync_info": {"on_update": [], "on_wait": [w]}})
                    si["on_wait"] = keep
                out.append(inst)
            b["instructions"] = out
    return json.dumps(j).encode()


"""Kernel strategy (per core, data-parallel over batch; 8 elems = 4 pairs):

- Residual x0 in fp32 [tokens, C]; post-attention residual x1 in bf16.
- Matmul activations transposed [C, 2, N] bf16 via PE transposes.
- LN affine folded into qkv/fc1; gamma1/2 folded into proj/fc2; attention
  scale folded into q weights; rel-pos bias applied as exp(rpb) multiply
  on DVE (PSUM logits stay pure q.k).
- Attention per head-pair hp: row-packed logits (2 heads concurrent on PE
  via 64-row tile_position), one exp per tk-tile covering 2 heads x 2
  elems, denominator via ones-column of V + batched Ln/Exp + gpsimd
  partition_broadcast, O copied to SBUF to free PSUM early.
- fc1 weights streamed from HBM per output-block (re-layout [ob,f,k,c]).
- bufs=2 on transposed-activation tiles + bufs=3 on residual tags so the
  next pair's LN1/qkT/v matmuls keep the PE dense during softmax.
"""

import numpy as np

import concourse.bass as bass
import concourse.tile as tile
import concourse.mybir as mybir
from concourse.masks import make_identity

FP32 = mybir.dt.float32
BF16 = mybir.dt.bfloat16

B = 64
N = 197
C = 768
H = 12
D = 64
HID = 3072
NCORES = 8
BPC = B // NCORES          # 8 batch elems per core
NPAIRS_FULL = BPC // 2     # 4
KT = C // 128              # 6 k-tiles of 128 over features
KT2 = HID // 128           # 24 k-tiles over hidden
LN_EPS = 1e-5

# token tiling: 197 = 128 + 69
T_TILES = [(0, 128), (128, 69)]
# output chunks over feature dim 768 = 512 + 256
C_CHUNKS = [(0, 512), (512, 256)]

AL = mybir.AluOpType
AF = mybir.ActivationFunctionType


def build_nc(npairs=NPAIRS_FULL):
    nb = 2 * npairs
    nc = bass.Bass()

    x_d = nc.dram_tensor("x", [nb, N, C], FP32, kind="ExternalInput")
    qkvT_d = nc.dram_tensor("qkvT", [C, 3 * C], BF16, kind="ExternalInput")
    projT_d = nc.dram_tensor("projT", [C, C], BF16, kind="ExternalInput")
    fc1s_d = nc.dram_tensor("fc1s", [KT2, 128, KT, 128], BF16, kind="ExternalInput")
    fc2T_d = nc.dram_tensor("fc2T", [HID, C], BF16, kind="ExternalInput")
    erpb0_d = nc.dram_tensor("erpb0", [128, H, N], BF16, kind="ExternalInput")
    erpb1_d = nc.dram_tensor("erpb1", [69, H, N], BF16, kind="ExternalInput")
    qb_d = nc.dram_tensor("qb", [C], FP32, kind="ExternalInput")
    kb_d = nc.dram_tensor("kb", [C], FP32, kind="ExternalInput")
    fc1b_d = nc.dram_tensor("fc1b", [HID], FP32, kind="ExternalInput")
    vb_d = nc.dram_tensor("vbrow", [C], BF16, kind="ExternalInput")
    pb_d = nc.dram_tensor("pbrow", [C], BF16, kind="ExternalInput")
    f2b_d = nc.dram_tensor("f2brow", [C], BF16, kind="ExternalInput")
    y_d = nc.dram_tensor("y", [nb, N, C], FP32, kind="ExternalOutput")

    with tile.TileContext(nc) as tc:
        with (
            tc.tile_pool(name="singles", bufs=1) as singles,
            tc.tile_pool(name="resid", bufs=3) as resid,     # x0 + out (fp32), rotating
            tc.tile_pool(name="x1p", bufs=1) as x1p,         # x1 bf16
            tc.tile_pool(name="b394", bufs=2) as b394,       # bf16 [128, 2, N] transposed acts
            tc.tile_pool(name="xn", bufs=2) as xnp,
            tc.tile_pool(name="vpool", bufs=2) as vpool,
            tc.tile_pool(name="fc1sp", bufs=3) as fc1sp,
            tc.tile_pool(name="etp", bufs=2) as etp,
            tc.tile_pool(name="ocpp", bufs=2) as ocpp,
            tc.tile_pool(name="small", bufs=8) as small,
            tc.tile_pool(name="ps_mm", bufs=2, space="PSUM") as ps_mm,
            tc.tile_pool(name="ps_l0", bufs=1, space="PSUM") as ps_l0,
            tc.tile_pool(name="ps_l1", bufs=1, space="PSUM") as ps_l1,
            tc.tile_pool(name="ps_o", bufs=1, space="PSUM") as ps_o,
        ):
            # ---- persistent weights / constants ----
            qkvT = [singles.tile([128, 3 * C], BF16, tag=f"qkvT{k}", name=f"qkvT{k}") for k in range(KT)]
            projT = [singles.tile([128, C], BF16, tag=f"projT{k}", name=f"projT{k}") for k in range(KT)]
            fc2T = [singles.tile([128, C], BF16, tag=f"fc2T{k}", name=f"fc2T{k}") for k in range(KT2)]
            erpb0 = singles.tile([128, H, N], BF16, tag="erpb0")
            erpb1 = singles.tile([69, H, N], BF16, tag="erpb1")
            qb_sb = singles.tile([128, KT], FP32, tag="qb")
            kb_sb = singles.tile([128, KT], FP32, tag="kb")
            fc1b_sb = singles.tile([128, KT2], FP32, tag="fc1b")
            brow3 = singles.tile([65, C], BF16, tag="brow3")
            vb_sb = brow3[0:1, :]
            pb_sb = brow3[32:33, :]
            f2b_sb = brow3[64:65, :]
            ident = singles.tile([128, 128], BF16, tag="ident")
            ones_col = singles.tile([65, 128], BF16, tag="ones")
            eps_sb = singles.tile([128, 1], FP32, tag="eps")

            for k in range(KT):
                nc.sync.dma_start(qkvT[k][:], qkvT_d[k * 128:(k + 1) * 128, :])
            for k in range(KT):
                nc.sync.dma_start(projT[k][:], projT_d[k * 128:(k + 1) * 128, :])
            for k in range(KT2):
                nc.sync.dma_start(fc2T[k][:], fc2T_d[k * 128:(k + 1) * 128, :])
            nc.sync.dma_start(erpb0[:], erpb0_d[:])
            nc.sync.dma_start(erpb1[:], erpb1_d[:])
            nc.sync.dma_start(qb_sb[:], qb_d[:].rearrange("(k p) -> p k", p=128))
            nc.sync.dma_start(kb_sb[:], kb_d[:].rearrange("(k p) -> p k", p=128))
            nc.sync.dma_start(fc1b_sb[:], fc1b_d[:].rearrange("(k p) -> p k", p=128))
            nc.sync.dma_start(brow3[0:1, :], vb_d[None, :])
            nc.sync.dma_start(brow3[32:33, :], pb_d[None, :])
            nc.sync.dma_start(brow3[64:65, :], f2b_d[None, :])
            make_identity(nc, ident[:])
            nc.vector.memset(ones_col[:], 1.0)
            nc.vector.memset(eps_sb[:], LN_EPS)

            def ln_transpose(x_tiles, tag, out_tags):
                """LN over feature dim + PE-transpose into [128, 2, N] bf16 tiles."""
                xT = [b394.tile([128, 2, N], BF16, tag=out_tags[k], name=f"{tag}T{k}")
                      for k in range(KT)]
                for (e, j), xt in x_tiles.items():
                    toff, tcnt = T_TILES[j]
                    stats = small.tile([128, 3, 6], FP32, tag=f"st_{tag}", bufs=4)
                    mv = small.tile([128, 2], FP32, tag=f"mv_{tag}")
                    sd = small.tile([128, 1], FP32, tag=f"sd_{tag}")
                    rstd = small.tile([128, 1], FP32, tag=f"rs_{tag}")
                    for g in range(3):
                        nc.vector.bn_stats(stats[:tcnt, g, :], xt[:tcnt, g * 256:(g + 1) * 256])
                    nc.vector.bn_aggr(mv[:tcnt], stats[:tcnt])
                    nc.scalar.activation(sd[:tcnt], mv[:tcnt, 1:2], AF.Ln, bias=eps_sb[:tcnt])
                    nc.scalar.activation(rstd[:tcnt], sd[:tcnt], AF.Exp, scale=-0.5)
                    xn = xnp.tile([128, C], BF16, tag="xn")
                    nc.vector.tensor_scalar(
                        xn[:tcnt, :], xt[:tcnt, :],
                        mv[:tcnt, 0:1], rstd[:tcnt, 0:1],
                        op0=AL.subtract, op1=AL.mult)
                    for cb in range(KT):
                        pt = ps_mm.tile([128, 512], BF16, tag="mm", name=f"tr_{tag}")
                        nc.tensor.transpose(
                            pt[:128, :tcnt],
                            xn[:tcnt, cb * 128:(cb + 1) * 128],
                            ident[:tcnt, :tcnt])
                        nc.vector.tensor_copy(
                            xT[cb][:, e, toff:toff + tcnt],
                            pt[:128, :tcnt])
                return xT

            for s in range(npairs):
                # ---------------- load x0 ----------------
                x0 = {}
                for e in range(2):
                    bidx = 2 * s + e
                    for j, (toff, tcnt) in enumerate(T_TILES):
                        t = resid.tile([128, C], FP32, tag=f"x0_{e}{j}", name=f"x0_{e}{j}")
                        nc.scalar.dma_start(t[:tcnt, :], x_d[bidx, toff:toff + tcnt, :])
                        x0[(e, j)] = t

                # ---------------- LN1 + transpose ----------------
                xnT = ln_transpose(x0, "ln1", [f"b394_xnT{k}" for k in range(KT)])

                # ---------------- qT, kT ----------------
                qT = [b394.tile([128, 2, N], BF16, tag=f"b394_qT{ob}", name=f"qT{ob}") for ob in range(KT)]
                kT = [b394.tile([128, 2, N], BF16, tag=f"b394_kT{ob}", name=f"kT{ob}", bufs=1) for ob in range(KT)]
                for dst, base, bias in ((qT, 0, qb_sb), (kT, C, kb_sb)):
                    for ob in range(KT):
                        ps = ps_mm.tile([128, 2, N], FP32, tag="mm")
                        for k in range(KT):
                            nc.tensor.matmul(
                                ps[:, :, :], qkvT[k][:, base + ob * 128: base + (ob + 1) * 128],
                                xnT[k][:, :, :], start=(k == 0), stop=(k == KT - 1))
                        nc.vector.tensor_scalar_add(dst[ob][:, :, :], ps[:, :, :], bias[:, ob:ob + 1])

                # ---------------- v (normal layout, per elem/t-tile) ----------------
                v_sb = {}
                for e in range(2):
                    for j, (toff, tcnt) in enumerate(T_TILES):
                        vt = vpool.tile([128, H, D + 1], BF16, tag=f"v{e}{j}", bufs=2 if e == 0 else 1)
                        nc.vector.memset(vt[:, :, D:D + 1], 1.0)
                        for ci, (coff, csz) in enumerate(C_CHUNKS):
                            ps = ps_mm.tile([128, 512], FP32, tag="mm")
                            for k in range(KT):
                                nc.tensor.matmul(
                                    ps[:tcnt, :csz],
                                    xnT[k][:, e, toff:toff + tcnt],
                                    qkvT[k][:, 2 * C + coff: 2 * C + coff + csz],
                                    start=(k == 0), stop=False)
                            nc.tensor.matmul(
                                ps[:tcnt, :csz],
                                ones_col[0:1, :tcnt],
                                vb_sb[:, coff:coff + csz],
                                start=False, stop=True)
                            h0 = coff // D
                            nh = csz // D
                            nc.vector.tensor_copy(
                                vt[:tcnt, h0:h0 + nh, 0:D],
                                ps[:tcnt, :csz])
                        v_sb[(e, j)] = vt

                # ---------------- attention ----------------
                aT = [b394.tile([128, 2, N], BF16, tag=f"b394_aT{cb}", name=f"aT{cb}") for cb in range(KT)]
                for hp in range(KT):
                    hA = 2 * hp
                    # logits: slot = 2*head_local + e; (A,e)->bank0, (B,e)->bank1
                    Lj0 = ps_l0.tile([128, 4, 256], FP32, tag="Lj0")
                    Lj1 = ps_l1.tile([69, 4, 256], FP32, tag="Lj1")
                    for e in range(2):
                        for jt, Lt, (tkoff, tkcnt) in ((0, Lj0, T_TILES[0]), (1, Lj1, T_TILES[1])):
                            for hl in range(2):
                                rbase = 64 * hl
                                sl = 2 * hl + e
                                nc.tensor.matmul(
                                    Lt[:tkcnt, sl, 0:N],
                                    kT[hp][rbase:rbase + 64, e, tkoff:tkoff + tkcnt],
                                    qT[hp][rbase:rbase + 64, e, :],
                                    start=True, stop=True)
                    # exp (one op per tk-tile covering 2 heads x 2 elems)
                    et0 = etp.tile([128, 4, N], BF16, tag="et0")
                    et1 = etp.tile([69, 4, N], BF16, tag="et1")
                    nc.scalar.activation(et0[:, :, :], Lj0[:, :, 0:N], AF.Exp)
                    nc.scalar.activation(et1[:69, :, :], Lj1[:69, :, 0:N], AF.Exp)
                    # multiply in exp(rel-pos-bias) (same bias for both elems)
                    for hl in range(2):
                        h = hA + hl
                        sl = slice(2 * hl, 2 * hl + 2)
                        nc.vector.tensor_tensor(
                            et0[:, sl, :], et0[:, sl, :],
                            erpb0[:, h:h + 1, :].broadcast_to([128, 2, N]), op=AL.mult)
                        nc.vector.tensor_tensor(
                            et1[:69, sl, :], et1[:69, sl, :],
                            erpb1[:69, h:h + 1, :].broadcast_to([69, 2, N]), op=AL.mult)
                    # O = [v; 1]^T @ et  -> [65, slot, N]
                    O = ps_o.tile([65, 4, 256], FP32, tag="O")
                    for e in range(2):
                        for hl in range(2):
                            h = hA + hl
                            sl = 2 * hl + e
                            nc.tensor.matmul(
                                O[:65, sl, 0:N],
                                v_sb[(e, 0)][:128, h, :],
                                et0[:128, sl, :], start=True, stop=False)
                            nc.tensor.matmul(
                                O[:65, sl, 0:N],
                                v_sb[(e, 1)][:69, h, :],
                                et1[:69, sl, :], start=False, stop=True)
                    # evacuate PSUM
                    ocp = ocpp.tile([65, 4, N], BF16, tag="ocp")
                    nc.vector.tensor_copy(ocp[:, :, :], O[:65, :, 0:N])
                    # denominator reciprocal rows: r = exp(-ln(den))
                    lden = small.tile([1, 4, N], BF16, tag="lden", bufs=2)
                    nc.scalar.activation(lden[:, :, :], ocp[64:65, :, :], AF.Ln)
                    nc.scalar.activation(lden[:, :, :], lden[:, :, :], AF.Exp, scale=-1.0)
                    # PE-broadcast reciprocal rows into the freed O slot
                    rbn = ps_o.tile([64, 4, 256], FP32, tag="O", name="rbn")
                    for hl in range(2):
                        nc.tensor.matmul(
                            rbn[0:64, 2 * hl:2 * hl + 2, 0:N],
                            ones_col[0:1, 0:64],
                            lden[0:1, 2 * hl:2 * hl + 2, :],
                            start=True, stop=True)
                    # normalize -> aT
                    for hl in range(2):
                        rbase = 64 * hl
                        sl = slice(2 * hl, 2 * hl + 2)
                        nc.vector.tensor_tensor(
                            aT[hp][rbase:rbase + 64, :, :],
                            ocp[0:64, sl, :], rbn[0:64, sl, 0:N], op=AL.mult)

                # ---------------- proj + residual -> x1 (bf16) ----------------
                x1 = {}
                for e in range(2):
                    for j, (toff, tcnt) in enumerate(T_TILES):
                        xt = x1p.tile([128, C], BF16, tag=f"x1_{e}{j}")
                        for ci, (coff, csz) in enumerate(C_CHUNKS):
                            ps = ps_mm.tile([128, 512], FP32, tag="mm")
                            for k in range(KT):
                                nc.tensor.matmul(
                                    ps[:tcnt, :csz],
                                    aT[k][:, e, toff:toff + tcnt],
                                    projT[k][:, coff:coff + csz],
                                    start=(k == 0), stop=False)
                            nc.tensor.matmul(
                                ps[:tcnt, :csz],
                                ones_col[32:33, :tcnt],
                                pb_sb[:, coff:coff + csz],
                                start=False, stop=True)
                            nc.vector.tensor_tensor(
                                xt[:tcnt, coff:coff + csz],
                                ps[:tcnt, :csz],
                                x0[(e, j)][:tcnt, coff:coff + csz], op=AL.add)
                        x1[(e, j)] = xt

                # ---------------- LN2 + transpose ----------------
                hnT = ln_transpose(x1, "ln2", [f"b394_hnT{k}" for k in range(KT)])

                # ---------------- fc1 (streamed weights) + gelu -> hT ----------------
                _ht_tags = ([f"b394_xnT{k}" for k in range(KT)] + [f"b394_qT{k}" for k in range(KT)]
                            + [f"b394_h{k}" for k in range(KT)] + [f"b394_aT{k}" for k in range(KT)])
                _ht_bufs = [2] * KT + [2] * KT + [1] * KT + [2] * KT
                hT = [b394.tile([128, 2, N], BF16, tag=_ht_tags[ob], name=f"hT{ob}", bufs=_ht_bufs[ob])
                      for ob in range(KT2)]
                for ob in range(KT2):
                    fst = fc1sp.tile([128, KT, 128], BF16, tag="fc1s")
                    nc.sync.dma_start(fst[:, :, :], fc1s_d[ob])
                    ps = ps_mm.tile([128, 2, N], FP32, tag="mm")
                    for k in range(KT):
                        nc.tensor.matmul(
                            ps[:, :, :], fst[:, k, :],
                            hnT[k][:, :, :], start=(k == 0), stop=(k == KT - 1))
                    nc.scalar.activation(
                        hT[ob][:, :, :], ps[:, :, :], AF.Gelu,
                        bias=fc1b_sb[:, ob:ob + 1])

                # ---------------- fc2 + residual -> y ----------------
                for e in range(2):
                    bidx = 2 * s + e
                    for j, (toff, tcnt) in enumerate(T_TILES):
                        ot = resid.tile([128, C], FP32, tag=f"x0_{e}{j}", name=f"out_{e}{j}")
                        for ci, (coff, csz) in enumerate(C_CHUNKS):
                            ps = ps_mm.tile([128, 512], FP32, tag="mm")
                            for k in range(KT2):
                                nc.tensor.matmul(
                                    ps[:tcnt, :csz],
                                    hT[k][:, e, toff:toff + tcnt],
                                    fc2T[k][:, coff:coff + csz],
                                    start=(k == 0), stop=False)
                            nc.tensor.matmul(
                                ps[:tcnt, :csz],
                                ones_col[64:65, :tcnt],
                                f2b_sb[:, coff:coff + csz],
                                start=False, stop=True)
                            nc.vector.tensor_tensor(
                                ot[:tcnt, coff:coff + csz],
                                ps[:tcnt, :csz],
                                x1[(e, j)][:tcnt, coff:coff + csz], op=AL.add)
                        nc.gpsimd.dma_start(y_d[bidx, toff:toff + tcnt, :], ot[:tcnt, :])

    return nc


def fold_weights(inputs):
    """Host-side folding. Returns dict of per-core-shared input arrays."""
    import ml_dtypes
    f32 = np.float32
    bf16 = ml_dtypes.bfloat16
    g = {k: np.asarray(v) for k, v in inputs.items()}
    n1w, n1b = g["n1_w"].astype(f32), g["n1_b"].astype(f32)
    n2w, n2b = g["n2_w"].astype(f32), g["n2_b"].astype(f32)
    g1, g2 = g["gamma1"].astype(f32), g["gamma2"].astype(f32)
    qkv_w = g["qkv_w"].astype(f32)
    q_bias, v_bias = g["q_bias"].astype(f32), g["v_bias"].astype(f32)
    proj_w, proj_b = g["proj_w"].astype(f32), g["proj_b"].astype(f32)
    fc1_w, fc1_b = g["fc1_w"].astype(f32), g["fc1_b"].astype(f32)
    fc2_w, fc2_b = g["fc2_w"].astype(f32), g["fc2_b"].astype(f32)

    qkv_bias = np.concatenate([q_bias, np.zeros_like(q_bias), v_bias])
    Wq = qkv_w * n1w[None, :]
    bq = qkv_bias + qkv_w @ n1b
    scale = (C // H) ** -0.5
    Wq[:C] *= scale
    bq[:C] *= scale

    Pw = g1[:, None] * proj_w
    pb = g1 * proj_b
    F1 = fc1_w * n2w[None, :]
    f1b = fc1_b + fc1_w @ n2b
    F2 = g2[:, None] * fc2_w
    f2b = g2 * fc2_b

    table = g["rel_bias_table"].astype(f32)
    idx = np.asarray(g["rel_index"]).reshape(-1)
    rpb_ref = table[idx].reshape(N, N, H).transpose(2, 0, 1)   # [h, tq, tk]
    erpbT = np.exp(rpb_ref.transpose(0, 2, 1))                 # [h, tk, tq]
    erpb0 = np.ascontiguousarray(erpbT[:, :128, :].transpose(1, 0, 2)).astype(bf16)
    erpb1 = np.ascontiguousarray(erpbT[:, 128:, :].transpose(1, 0, 2)).astype(bf16)

    F1T = np.ascontiguousarray(F1.T)                           # [C, HID]
    fc1s = np.ascontiguousarray(
        F1T.reshape(KT, 128, KT2, 128).transpose(2, 1, 0, 3)).astype(bf16)

    return {
        "qkvT": np.ascontiguousarray(Wq.T).astype(bf16),
        "projT": np.ascontiguousarray(Pw.T).astype(bf16),
        "fc1s": fc1s,
        "fc2T": np.ascontiguousarray(F2.T).astype(bf16),
        "erpb0": erpb0,
        "erpb1": erpb1,
        "qb": np.ascontiguousarray(bq[:C]),
        "kb": np.ascontiguousarray(bq[C:2 * C]),
        "fc1b": f1b,
        "vbrow": bq[2 * C:].astype(bf16),
        "pbrow": pb.astype(bf16),
        "f2brow": f2b.astype(bf16),
    }


_CACHE = {}


def _get_nc():
    if "nc" not in _CACHE:
        nc = build_nc()
        patched = _legalize_waits(nc.to_json_bytes())
        nc.to_json_bytes = lambda: patched
        _CACHE["nc"] = nc
    return _CACHE["nc"]


def kernel(**inputs):
    from concourse.bass_utils import run_bass_kernel_spmd
    nc = _get_nc()
    folded = fold_weights(inputs)
    x = np.ascontiguousarray(np.asarray(inputs["x"], dtype=np.float32))
    assert x.shape == (B, N, C), x.shape
    in_maps = []
    for c in range(NCORES):
        m = dict(folded)
        m["x"] = np.ascontiguousarray(x[c * BPC:(c + 1) * BPC])
        in_maps.append(m)
    res = run_bass_kernel_spmd(nc, in_maps, core_ids=list(range(NCORES)))
    out = np.concatenate([res.results[c]["y"] for c in range(NCORES)], axis=0)
    return out.astype(np.float32)


# revision 11
# speedup vs baseline: 1.4539x; 1.1593x over previous
"""Trainium2 Bass kernel for nn_Block_74363063763569 (BEiT-style transformer block).

Data-parallel over batch across 8 NeuronCores (8 elems/core), zero collectives.
Self-contained: builds, compiles (cached) and runs the Bass kernel via
run_bass_kernel_spmd on cores 0-7.
"""
import sys, json
sys.path.insert(0, "/opt/trn_rl_repo")
import numpy as np


def _legalize_waits(bir_bytes, max_waits=1):
    """This container's walrus rejects >1 sync wait per instruction; split
    extras into preceding single-wait EventSemaphore instructions."""
    j = json.loads(bir_bytes)
    for f in j["functions"]:
        for b in f["blocks"]:
            out = []
            for inst in b["instructions"]:
                si = inst.get("sync_info")
                waits = si.get("on_wait", []) if si else []
                if len(waits) > max_waits:
                    keep, extra = waits[:max_waits], waits[max_waits:]
                    for k, w in enumerate(extra):
                        out.append({"debug": inst.get("debug", 0), "engine": inst["engine"],
                                    "ins": [], "name": f"{inst['name']}_w{k}",
                                    "opcode": "EventSemaphore", "outs": [],
                                    "sync_info": {"on_update": [], "on_wait": [w]}})
                    si["on_wait"] = keep
                out.append(inst)
            b["instructions"] = out
    return json.dumps(j).encode()


"""Kernel strategy (per core, data-parallel over batch; 8 elems = 4 pairs):

- Residual x0 in fp32 [tokens, C]; post-attention residual x1 in bf16.
- Matmul activations transposed [C, 2, N] bf16 via PE transposes.
- LN affine folded into qkv/fc1; gamma1/2 folded into proj/fc2; attention
  scale folded into q weights; rel-pos bias applied as exp(rpb) multiply
  on DVE (PSUM logits stay pure q.k).
- Attention per head-pair hp: row-packed logits (2 heads concurrent on PE
  via 64-row tile_position), one exp per tk-tile covering 2 heads x 2
  elems, denominator via ones-column of V + batched Ln/Exp + gpsimd
  partition_broadcast, O copied to SBUF to free PSUM early.
- fc1 weights streamed from HBM per output-block (re-layout [ob,f,k,c]).
- bufs=2 on transposed-activation tiles + bufs=3 on residual tags so the
  next pair's LN1/qkT/v matmuls keep the PE dense during softmax.
"""

import numpy as np

import concourse.bass as bass
import concourse.tile as tile
import concourse.mybir as mybir
from concourse.masks import make_identity

FP32 = mybir.dt.float32
BF16 = mybir.dt.bfloat16

B = 64
N = 197
C = 768
H = 12
D = 64
HID = 3072
NCORES = 8
BPC = B // NCORES          # 8 batch elems per core
NPAIRS_FULL = BPC // 2     # 4
KT = C // 128              # 6 k-tiles of 128 over features
KT2 = HID // 128           # 24 k-tiles over hidden
LN_EPS = 1e-5

# token tiling: 197 = 128 + 69
T_TILES = [(0, 128), (128, 69)]
# output chunks over feature dim 768 = 512 + 256
C_CHUNKS = [(0, 512), (512, 256)]

AL = mybir.AluOpType
AF = mybir.ActivationFunctionType


def build_nc(npairs=NPAIRS_FULL):
    nb = 2 * npairs
    nc = bass.Bass()

    x_d = nc.dram_tensor("x", [nb, N, C], FP32, kind="ExternalInput")
    qkvT_d = nc.dram_tensor("qkvT", [C, 3 * C], BF16, kind="ExternalInput")
    projT_d = nc.dram_tensor("projT", [C, C], BF16, kind="ExternalInput")
    fc1s_d = nc.dram_tensor("fc1s", [KT2, 128, KT, 128], BF16, kind="ExternalInput")
    fc2T_d = nc.dram_tensor("fc2T", [HID, C], BF16, kind="ExternalInput")
    erpb0_d = nc.dram_tensor("erpb0", [128, H, N], BF16, kind="ExternalInput")
    erpb1_d = nc.dram_tensor("erpb1", [69, H, N], BF16, kind="ExternalInput")
    qb_d = nc.dram_tensor("qb", [C], FP32, kind="ExternalInput")
    kb_d = nc.dram_tensor("kb", [C], FP32, kind="ExternalInput")
    fc1b_d = nc.dram_tensor("fc1b", [HID], FP32, kind="ExternalInput")
    vb_d = nc.dram_tensor("vbrow", [C], BF16, kind="ExternalInput")
    pb_d = nc.dram_tensor("pbrow", [C], BF16, kind="ExternalInput")
    f2b_d = nc.dram_tensor("f2brow", [C], BF16, kind="ExternalInput")
    y_d = nc.dram_tensor("y", [nb, N, C], FP32, kind="ExternalOutput")

    with tile.TileContext(nc) as tc:
        with (
            tc.tile_pool(name="singles", bufs=1) as singles,
            tc.tile_pool(name="resid", bufs=3) as resid,     # x0 + out (fp32), rotating
            tc.tile_pool(name="x1p", bufs=1) as x1p,         # x1 bf16
            tc.tile_pool(name="b394", bufs=2) as b394,       # bf16 [128, 2, N] transposed acts
            tc.tile_pool(name="xn", bufs=2) as xnp,
            tc.tile_pool(name="vpool", bufs=2) as vpool,
            tc.tile_pool(name="fc1sp", bufs=3) as fc1sp,
            tc.tile_pool(name="etp", bufs=2) as etp,
            tc.tile_pool(name="ocpp", bufs=2) as ocpp,
            tc.tile_pool(name="small", bufs=8) as small,
            tc.tile_pool(name="ps_mm", bufs=2, space="PSUM") as ps_mm,
            tc.tile_pool(name="ps_l0", bufs=1, space="PSUM") as ps_l0,
            tc.tile_pool(name="ps_l1", bufs=1, space="PSUM") as ps_l1,
            tc.tile_pool(name="ps_o", bufs=1, space="PSUM") as ps_o,
        ):
            # ---- persistent weights / constants ----
            qkvT = [singles.tile([128, 3 * C], BF16, tag=f"qkvT{k}", name=f"qkvT{k}") for k in range(KT)]
            projT = [singles.tile([128, C], BF16, tag=f"projT{k}", name=f"projT{k}") for k in range(KT)]
            fc2T = [singles.tile([128, C], BF16, tag=f"fc2T{k}", name=f"fc2T{k}") for k in range(KT2)]
            erpb0 = singles.tile([128, H, N], BF16, tag="erpb0")
            erpb1 = singles.tile([69, H, N], BF16, tag="erpb1")
            qb_sb = singles.tile([128, KT], FP32, tag="qb")
            kb_sb = singles.tile([128, KT], FP32, tag="kb")
            fc1b_sb = singles.tile([128, KT2], FP32, tag="fc1b")
            brow3 = singles.tile([65, C], BF16, tag="brow3")
            vb_sb = brow3[0:1, :]
            pb_sb = brow3[32:33, :]
            f2b_sb = brow3[64:65, :]
            ident = singles.tile([128, 128], BF16, tag="ident")
            ones_col = singles.tile([65, 128], BF16, tag="ones")
            eps_sb = singles.tile([128, 1], FP32, tag="eps")

            for k in range(KT):
                nc.sync.dma_start(qkvT[k][:], qkvT_d[k * 128:(k + 1) * 128, :])
            for k in range(KT):
                nc.sync.dma_start(projT[k][:], projT_d[k * 128:(k + 1) * 128, :])
            for k in range(KT2):
                nc.sync.dma_start(fc2T[k][:], fc2T_d[k * 128:(k + 1) * 128, :])
            nc.sync.dma_start(erpb0[:], erpb0_d[:])
            nc.sync.dma_start(erpb1[:], erpb1_d[:])
            nc.sync.dma_start(qb_sb[:], qb_d[:].rearrange("(k p) -> p k", p=128))
            nc.sync.dma_start(kb_sb[:], kb_d[:].rearrange("(k p) -> p k", p=128))
            nc.sync.dma_start(fc1b_sb[:], fc1b_d[:].rearrange("(k p) -> p k", p=128))
            nc.sync.dma_start(brow3[0:1, :], vb_d[None, :])
            nc.sync.dma_start(brow3[32:33, :], pb_d[None, :])
            nc.sync.dma_start(brow3[64:65, :], f2b_d[None, :])
            make_identity(nc, ident[:])
            nc.vector.memset(ones_col[:], 1.0)
            nc.vector.memset(eps_sb[:], LN_EPS)

            def ln_transpose(x_tiles, tag, out_tags):
                """LN over feature dim + PE-transpose into [128, 2, N] bf16 tiles."""
                xT = [b394.tile([128, 2, N], BF16, tag=out_tags[k], name=f"{tag}T{k}")
                      for k in range(KT)]
                for (e, j), xt in x_tiles.items():
                    toff, tcnt = T_TILES[j]
                    stats = small.tile([128, 3, 6], FP32, tag=f"st_{tag}", bufs=4)
                    mv = small.tile([128, 2], FP32, tag=f"mv_{tag}")
                    sd = small.tile([128, 1], FP32, tag=f"sd_{tag}")
                    rstd = small.tile([128, 1], FP32, tag=f"rs_{tag}")
                    for g in range(3):
                        nc.vector.bn_stats(stats[:tcnt, g, :], xt[:tcnt, g * 256:(g + 1) * 256])
                    nc.vector.bn_aggr(mv[:tcnt], stats[:tcnt])
                    nc.scalar.activation(sd[:tcnt], mv[:tcnt, 1:2], AF.Ln, bias=eps_sb[:tcnt])
                    nc.scalar.activation(rstd[:tcnt], sd[:tcnt], AF.Exp, scale=-0.5)
                    xn = xnp.tile([128, C], BF16, tag="xn")
                    nc.vector.tensor_scalar(
                        xn[:tcnt, :], xt[:tcnt, :],
                        mv[:tcnt, 0:1], rstd[:tcnt, 0:1],
                        op0=AL.subtract, op1=AL.mult)
                    for cb in range(KT):
                        pt = ps_mm.tile([128, 512], BF16, tag="mm", name=f"tr_{tag}")
                        nc.tensor.transpose(
                            pt[:128, :tcnt],
                            xn[:tcnt, cb * 128:(cb + 1) * 128],
                            ident[:tcnt, :tcnt])
                        nc.vector.tensor_copy(
                            xT[cb][:, e, toff:toff + tcnt],
                            pt[:128, :tcnt])
                return xT

            def ln1_unit(fs, e, j):
                xt = fs['x0'][(e, j)]
                xT = fs['xnT']
                toff, tcnt = T_TILES[j]
                stats = small.tile([128, 3, 6], FP32, tag="st_ln1", bufs=4)
                mv = small.tile([128, 2], FP32, tag="mv_ln1")
                sd = small.tile([128, 1], FP32, tag="sd_ln1")
                rstd = small.tile([128, 1], FP32, tag="rs_ln1")
                for g in range(3):
                    nc.vector.bn_stats(stats[:tcnt, g, :], xt[:tcnt, g * 256:(g + 1) * 256])
                nc.vector.bn_aggr(mv[:tcnt], stats[:tcnt])
                nc.scalar.activation(sd[:tcnt], mv[:tcnt, 1:2], AF.Ln, bias=eps_sb[:tcnt])
                nc.scalar.activation(rstd[:tcnt], sd[:tcnt], AF.Exp, scale=-0.5)
                xn = xnp.tile([128, C], BF16, tag="xn")
                nc.vector.tensor_scalar(
                    xn[:tcnt, :], xt[:tcnt, :],
                    mv[:tcnt, 0:1], rstd[:tcnt, 0:1],
                    op0=AL.subtract, op1=AL.mult)
                for cb in range(KT):
                    pt = ps_mm.tile([128, 512], BF16, tag="mm", name="tr_ln1")
                    nc.tensor.transpose(
                        pt[:128, :tcnt],
                        xn[:tcnt, cb * 128:(cb + 1) * 128],
                        ident[:tcnt, :tcnt])
                    nc.vector.tensor_copy(
                        xT[cb][:, e, toff:toff + tcnt],
                        pt[:128, :tcnt])

            def qk_obs(fs, which, obs):
                dst = fs['qT'] if which == 'q' else fs['kT']
                base = 0 if which == 'q' else C
                bias = qb_sb if which == 'q' else kb_sb
                xnT = fs['xnT']
                for ob in obs:
                    ps = ps_mm.tile([128, 2, N], FP32, tag="mm")
                    for k in range(KT):
                        nc.tensor.matmul(
                            ps[:, :, :], qkvT[k][:, base + ob * 128: base + (ob + 1) * 128],
                            xnT[k][:, :, :], start=(k == 0), stop=(k == KT - 1))
                    nc.vector.tensor_scalar_add(dst[ob][:, :, :], ps[:, :, :], bias[:, ob:ob + 1])

            def v_unit(fs, e, j):
                toff, tcnt = T_TILES[j]
                xnT = fs['xnT']
                vt = vpool.tile([128, H, D + 1], BF16, tag=f"v{e}{j}", bufs=2 if e == 0 else 1)
                nc.vector.memset(vt[:, :, D:D + 1], 1.0)
                for ci, (coff, csz) in enumerate(C_CHUNKS):
                    ps = ps_mm.tile([128, 512], FP32, tag="mm")
                    for k in range(KT):
                        nc.tensor.matmul(
                            ps[:tcnt, :csz],
                            xnT[k][:, e, toff:toff + tcnt],
                            qkvT[k][:, 2 * C + coff: 2 * C + coff + csz],
                            start=(k == 0), stop=False)
                    nc.tensor.matmul(
                        ps[:tcnt, :csz],
                        ones_col[0:1, :tcnt],
                        vb_sb[:, coff:coff + csz],
                        start=False, stop=True)
                    h0 = coff // D
                    nh = csz // D
                    nc.vector.tensor_copy(
                        vt[:tcnt, h0:h0 + nh, 0:D],
                        ps[:tcnt, :csz])
                fs['v'][(e, j)] = vt

            def make_front(s):
                fs = {'v': {}}

                def c0():
                    fs['x0'] = {}
                    for e in range(2):
                        bidx = 2 * s + e
                        for j, (toff, tcnt) in enumerate(T_TILES):
                            t = resid.tile([128, C], FP32, tag=f"x0_{e}{j}", name=f"x0_{e}{j}_{s}")
                            nc.scalar.dma_start(t[:tcnt, :], x_d[bidx, toff:toff + tcnt, :])
                            fs['x0'][(e, j)] = t
                    fs['xnT'] = [b394.tile([128, 2, N], BF16, tag=f"b394_xnT{k}", name=f"xnT{k}_{s}")
                                 for k in range(KT)]
                    ln1_unit(fs, 0, 0)

                def c1():
                    ln1_unit(fs, 0, 1)
                    ln1_unit(fs, 1, 0)

                def c2():
                    ln1_unit(fs, 1, 1)
                    fs['qT'] = [b394.tile([128, 2, N], BF16, tag=f"b394_qT{ob}", name=f"qT{ob}_{s}")
                                for ob in range(KT)]
                    fs['kT'] = [b394.tile([128, 2, N], BF16, tag=f"b394_kT{ob}", name=f"kT{ob}_{s}", bufs=1)
                                for ob in range(KT)]

                def c3():
                    qk_obs(fs, 'q', range(KT))

                def c4():
                    qk_obs(fs, 'k', range(KT))

                def c5():
                    for e in range(2):
                        for j in range(2):
                            v_unit(fs, e, j)

                return fs, [c0, c1, c2, c3, c4, c5]

            def attn_hp(fs, s, hp):
                qT, kT, v_sb, aT = fs['qT'], fs['kT'], fs['v'], fs['aT']
                hA = 2 * hp
                Lj0 = ps_l0.tile([128, 4, 256], FP32, tag="Lj0")
                Lj1 = ps_l1.tile([69, 4, 256], FP32, tag="Lj1")
                for e in range(2):
                    for jt, Lt, (tkoff, tkcnt) in ((0, Lj0, T_TILES[0]), (1, Lj1, T_TILES[1])):
                        for hl in range(2):
                            rbase = 64 * hl
                            sl = 2 * hl + e
                            nc.tensor.matmul(
                                Lt[:tkcnt, sl, 0:N],
                                kT[hp][rbase:rbase + 64, e, tkoff:tkoff + tkcnt],
                                qT[hp][rbase:rbase + 64, e, :],
                                start=True, stop=True)
                et0 = etp.tile([128, 4, N], BF16, tag="et0")
                et1 = etp.tile([69, 4, N], BF16, tag="et1")
                nc.scalar.activation(et0[:, :, :], Lj0[:, :, 0:N], AF.Exp)
                nc.scalar.activation(et1[:69, :, :], Lj1[:69, :, 0:N], AF.Exp)
                for hl in range(2):
                    h = hA + hl
                    sl = slice(2 * hl, 2 * hl + 2)
                    nc.vector.tensor_tensor(
                        et0[:, sl, :], et0[:, sl, :],
                        erpb0[:, h:h + 1, :].broadcast_to([128, 2, N]), op=AL.mult)
                    nc.vector.tensor_tensor(
                        et1[:69, sl, :], et1[:69, sl, :],
                        erpb1[:69, h:h + 1, :].broadcast_to([69, 2, N]), op=AL.mult)
                O = ps_o.tile([65, 4, 256], FP32, tag="O")
                for e in range(2):
                    for hl in range(2):
                        h = hA + hl
                        sl = 2 * hl + e
                        nc.tensor.matmul(
                            O[:65, sl, 0:N],
                            v_sb[(e, 0)][:128, h, :],
                            et0[:128, sl, :], start=True, stop=False)
                        nc.tensor.matmul(
                            O[:65, sl, 0:N],
                            v_sb[(e, 1)][:69, h, :],
                            et1[:69, sl, :], start=False, stop=True)
                ocp = ocpp.tile([65, 4, N], BF16, tag="ocp")
                nc.vector.tensor_copy(ocp[:, :, :], O[:65, :, 0:N])
                lden = small.tile([1, 4, N], BF16, tag="lden", bufs=2)
                nc.scalar.activation(lden[:, :, :], ocp[64:65, :, :], AF.Ln)
                nc.scalar.activation(lden[:, :, :], lden[:, :, :], AF.Exp, scale=-1.0)
                rbn = ps_o.tile([64, 4, 256], FP32, tag="O", name="rbn")
                for hl in range(2):
                    nc.tensor.matmul(
                        rbn[0:64, 2 * hl:2 * hl + 2, 0:N],
                        ones_col[0:1, 0:64],
                        lden[0:1, 2 * hl:2 * hl + 2, :],
                        start=True, stop=True)
                for hl in range(2):
                    rbase = 64 * hl
                    sl = slice(2 * hl, 2 * hl + 2)
                    nc.vector.tensor_tensor(
                        aT[hp][rbase:rbase + 64, :, :],
                        ocp[0:64, sl, :], rbn[0:64, sl, 0:N], op=AL.mult)

            def back(fs, s):
                aT, x0 = fs['aT'], fs['x0']
                # proj + residual -> x1 (bf16)
                x1 = {}
                for e in range(2):
                    for j, (toff, tcnt) in enumerate(T_TILES):
                        xt = x1p.tile([128, C], BF16, tag=f"x1_{e}{j}")
                        for ci, (coff, csz) in enumerate(C_CHUNKS):
                            ps = ps_mm.tile([128, 512], FP32, tag="mm")
                            for k in range(KT):
                                nc.tensor.matmul(
                                    ps[:tcnt, :csz],
                                    aT[k][:, e, toff:toff + tcnt],
                                    projT[k][:, coff:coff + csz],
                                    start=(k == 0), stop=False)
                            nc.tensor.matmul(
                                ps[:tcnt, :csz],
                                ones_col[32:33, :tcnt],
                                pb_sb[:, coff:coff + csz],
                                start=False, stop=True)
                            nc.vector.tensor_tensor(
                                xt[:tcnt, coff:coff + csz],
                                ps[:tcnt, :csz],
                                x0[(e, j)][:tcnt, coff:coff + csz], op=AL.add)
                        x1[(e, j)] = xt

                # LN2 + transpose
                hnT = ln_transpose(x1, "ln2", [f"b394_hnT{k}" for k in range(KT)])

                # fc1 (streamed weights) + gelu -> hT
                _ht_tags = ([f"b394_xnT{k}" for k in range(KT)] + [f"b394_qT{k}" for k in range(KT)]
                            + [f"b394_h{k}" for k in range(KT)] + [f"b394_aT{k}" for k in range(KT)])
                _ht_bufs = [2] * KT + [2] * KT + [1] * KT + [2] * KT
                hT = [b394.tile([128, 2, N], BF16, tag=_ht_tags[ob], name=f"hT{ob}_{s}", bufs=_ht_bufs[ob])
                      for ob in range(KT2)]
                for ob in range(KT2):
                    fst = fc1sp.tile([128, KT, 128], BF16, tag="fc1s")
                    nc.sync.dma_start(fst[:, :, :], fc1s_d[ob])
                    ps = ps_mm.tile([128, 2, N], FP32, tag="mm")
                    for k in range(KT):
                        nc.tensor.matmul(
                            ps[:, :, :], fst[:, k, :],
                            hnT[k][:, :, :], start=(k == 0), stop=(k == KT - 1))
                    nc.scalar.activation(
                        hT[ob][:, :, :], ps[:, :, :], AF.Gelu,
                        bias=fc1b_sb[:, ob:ob + 1])

                # fc2 + residual -> y
                for e in range(2):
                    bidx = 2 * s + e
                    for j, (toff, tcnt) in enumerate(T_TILES):
                        ot = resid.tile([128, C], FP32, tag=f"x0_{e}{j}", name=f"out_{e}{j}_{s}")
                        for ci, (coff, csz) in enumerate(C_CHUNKS):
                            ps = ps_mm.tile([128, 512], FP32, tag="mm")
                            for k in range(KT2):
                                nc.tensor.matmul(
                                    ps[:tcnt, :csz],
                                    hT[k][:, e, toff:toff + tcnt],
                                    fc2T[k][:, coff:coff + csz],
                                    start=(k == 0), stop=False)
                            nc.tensor.matmul(
                                ps[:tcnt, :csz],
                                ones_col[64:65, :tcnt],
                                f2b_sb[:, coff:coff + csz],
                                start=False, stop=True)
                            nc.vector.tensor_tensor(
                                ot[:tcnt, coff:coff + csz],
                                ps[:tcnt, :csz],
                                x1[(e, j)][:tcnt, coff:coff + csz], op=AL.add)
                        nc.gpsimd.dma_start(y_d[bidx, toff:toff + tcnt, :], ot[:tcnt, :])

            # ---- software-pipelined main loop ----
            fs0, chunks0 = make_front(0)
            for c in chunks0:
                c()
            fronts = {0: fs0}
            for s in range(npairs):
                fs = fronts[s]
                fs['aT'] = [b394.tile([128, 2, N], BF16, tag=f"b394_aT{cb}", name=f"aT{cb}_{s}")
                            for cb in range(KT)]
                nchunks = None
                if s + 1 < npairs:
                    fronts[s + 1], nchunks = make_front(s + 1)
                for hp in range(KT):
                    attn_hp(fs, s, hp)
                    if nchunks is not None:
                        nchunks[hp]()
                back(fs, s)
                del fronts[s]

    return nc


def fold_weights(inputs):
    """Host-side folding. Returns dict of per-core-shared input arrays."""
    import ml_dtypes
    f32 = np.float32
    bf16 = ml_dtypes.bfloat16
    g = {k: np.asarray(v) for k, v in inputs.items()}
    n1w, n1b = g["n1_w"].astype(f32), g["n1_b"].astype(f32)
    n2w, n2b = g["n2_w"].astype(f32), g["n2_b"].astype(f32)
    g1, g2 = g["gamma1"].astype(f32), g["gamma2"].astype(f32)
    qkv_w = g["qkv_w"].astype(f32)
    q_bias, v_bias = g["q_bias"].astype(f32), g["v_bias"].astype(f32)
    proj_w, proj_b = g["proj_w"].astype(f32), g["proj_b"].astype(f32)
    fc1_w, fc1_b = g["fc1_w"].astype(f32), g["fc1_b"].astype(f32)
    fc2_w, fc2_b = g["fc2_w"].astype(f32), g["fc2_b"].astype(f32)

    qkv_bias = np.concatenate([q_bias, np.zeros_like(q_bias), v_bias])
    Wq = qkv_w * n1w[None, :]
    bq = qkv_bias + qkv_w @ n1b
    scale = (C // H) ** -0.5
    Wq[:C] *= scale
    bq[:C] *= scale

    Pw = g1[:, None] * proj_w
    pb = g1 * proj_b
    F1 = fc1_w * n2w[None, :]
    f1b = fc1_b + fc1_w @ n2b
    F2 = g2[:, None] * fc2_w
    f2b = g2 * fc2_b

    table = g["rel_bias_table"].astype(f32)
    idx = np.asarray(g["rel_index"]).reshape(-1)
    rpb_ref = table[idx].reshape(N, N, H).transpose(2, 0, 1)   # [h, tq, tk]
    erpbT = np.exp(rpb_ref.transpose(0, 2, 1))                 # [h, tk, tq]
    erpb0 = np.ascontiguousarray(erpbT[:, :128, :].transpose(1, 0, 2)).astype(bf16)
    erpb1 = np.ascontiguousarray(erpbT[:, 128:, :].transpose(1, 0, 2)).astype(bf16)

    F1T = np.ascontiguousarray(F1.T)                           # [C, HID]
    fc1s = np.ascontiguousarray(
        F1T.reshape(KT, 128, KT2, 128).transpose(2, 1, 0, 3)).astype(bf16)

    return {
        "qkvT": np.ascontiguousarray(Wq.T).astype(bf16),
        "projT": np.ascontiguousarray(Pw.T).astype(bf16),
        "fc1s": fc1s,
        "fc2T": np.ascontiguousarray(F2.T).astype(bf16),
        "erpb0": erpb0,
        "erpb1": erpb1,
        "qb": np.ascontiguousarray(bq[:C]),
        "kb": np.ascontiguousarray(bq[C:2 * C]),
        "fc1b": f1b,
        "vbrow": bq[2 * C:].astype(bf16),
        "pbrow": pb.astype(bf16),
        "f2brow": f2b.astype(bf16),
    }


_CACHE = {}


def _get_nc():
    if "nc" not in _CACHE:
        nc = build_nc()
        patched = _legalize_waits(nc.to_json_bytes())
        nc.to_json_bytes = lambda: patched
        _CACHE["nc"] = nc
    return _CACHE["nc"]


def kernel(**inputs):
    from concourse.bass_utils import run_bass_kernel_spmd
    nc = _get_nc()
    folded = fold_weights(inputs)
    x = np.ascontiguousarray(np.asarray(inputs["x"], dtype=np.float32))
    assert x.shape == (B, N, C), x.shape
    in_maps = []
    for c in range(NCORES):
        m = dict(folded)
        m["x"] = np.ascontiguousarray(x[c * BPC:(c + 1) * BPC])
        in_maps.append(m)
    res = run_bass_kernel_spmd(nc, in_maps, core_ids=list(range(NCORES)))
    out = np.concatenate([res.results[c]["y"] for c in range(NCORES)], axis=0)
    return out.astype(np.float32)
